# revision 1
# baseline (speedup 1.0000x reference)
"""GATConv (nn_GATConv_45595372814934) Trainium2 Bass kernel, 8 NeuronCores.

kernel(**inputs) -> [100000, 1, 64] float32.

Strategy (graph/edge parallelism):
- Node/edge shard: core c owns nodes [12500c, 12500(c+1)) and their 16
  out-edges each (src is repeat(arange(N), 16), so edges are contiguous).
- Phase 1 (per core): support shard = x_c @ W' where W' = [W | W@a_dst |
  W@a_src], fp16 rows [support(64) | s_dst | s_src] -> AllGather into a
  full per-core fp16 table [100352, 66] in HBM.
- Phase 2 (per core): node n = s*128 + p (partition p); its 16 edges sit
  along the free dim.  One indirect DMA per PAIR of 128-node super-tiles
  gathers 4096 table rows (by dst) into SBUF; per-edge weight
  exp(lrelu(s_src + s_dst) - ln(deg)) via per-partition tensor_scalar +
  Exp activation with bias; weighted sum over the 16 edges is a vector
  multiply + strided reduce (no matmuls, no PSUM in phase 2).
"""

import os
import sys

sys.path.insert(0, "/opt/trn_rl_repo")

import numpy as np

import concourse.bacc as bacc
import concourse.bass as bass
import concourse.mybir as mybir
import concourse.tile as tile
from concourse.bass import AP
from concourse import bass_utils

F32 = mybir.dt.float32
F16 = mybir.dt.float16
I32 = mybir.dt.int32

N_NODES = 100000
IN_CH = 256
C = 64
DEG = 16
NEG_SLOPE = 0.2
NCORES = 8
NPC = N_NODES // NCORES          # 12500 real nodes per core
NPAD = -(-NPC // 128) * 128      # 12544
SUP = NPAD // 128                # 98 super-tiles
PAIRS = SUP // 2                 # 49 gather iterations (2 super-tiles each)
NROWS = NCORES * NPAD            # table rows
TW = 66                          # table row: support(64) | s_dst | s_src

LAST_EXEC_NS = None
_CACHED_NC = None


def _mkap(base: AP, extra_off: int, dims) -> AP:
    return AP(base.tensor, base.offset + extra_off,
              [list(base.ap[0])] + [list(d) for d in dims])


def _build_nc():
    nc = bacc.Bacc("TRN2", target_bir_lowering=False, debug=False,
                   num_devices=NCORES, num_swdge_queues=4)

    xT_d = nc.dram_tensor("xT", [IN_CH, NPAD], F16, kind="ExternalInput")
    dstT_d = nc.dram_tensor("dstT", [128, SUP * DEG], I32, kind="ExternalInput")
    nlnd_d = nc.dram_tensor("nlnd", [128, SUP], F32, kind="ExternalInput")
    wp_d = nc.dram_tensor("wp", [IN_CH, TW], F16, kind="ExternalInput")
    out_d = nc.dram_tensor("out", [NPAD, C], F32, kind="ExternalOutput")

    from concourse.replica_groups import maybe_share_collective_output_space
    aspace = maybe_share_collective_output_space(
        "AllGather", [list(range(NCORES))])
    shard_d = nc.dram_tensor("shard", [NPAD, TW], F16, kind="Internal")
    table_d = nc.dram_tensor("table", [NROWS, TW], F16, kind="Internal",
                             addr_space=aspace)

    dst_sb = nc.alloc_sbuf_tensor("dst_sb", [128, SUP * DEG], I32)
    ssrc_sb = nc.alloc_sbuf_tensor("ssrc_sb", [128, SUP], F32)
    nlnd_sb = nc.alloc_sbuf_tensor("nlnd_sb", [128, SUP], F32)
    wp_sb = nc.alloc_sbuf_tensor("wp_sb", [128, 2 * TW], F16)

    with tile.TileContext(nc) as tc:
        with (
            tc.tile_pool(name="xp", bufs=3) as xp,
            tc.tile_pool(name="stp", bufs=3) as stp,
            tc.tile_pool(name="gp", bufs=4) as gp,
            tc.tile_pool(name="pp", bufs=3) as pp,
            tc.tile_pool(name="sp", bufs=3) as sp,
            tc.tile_pool(name="obp", bufs=3) as obp,
            tc.tile_pool(name="ps1", bufs=2, space="PSUM") as ps1,
        ):
            nc.sync.dma_start(dst_sb.ap(), dstT_d.ap())
            nc.sync.dma_start(nlnd_sb.ap(), nlnd_d.ap())
            nc.sync.dma_start(
                wp_sb.ap(), wp_d.ap().rearrange("(a p) c -> p a c", p=128))
            wp3 = wp_sb.ap().rearrange("p (a c) -> p a c", c=TW)

            # phase 1: support table shard
            xT3 = xT_d.ap().rearrange("(a p) n -> p a n", p=128)
            for s in range(SUP):
                xt = xp.tile([128, 2, 128], F16, tag="xt")
                nc.sync.dma_start(xt[:], xT3[:, :, 128 * s:128 * (s + 1)])
                ps = ps1.tile([128, TW], F32, tag="ps1")
                nc.tensor.matmul(out=ps[:], lhsT=xt[:, 0, :], rhs=wp3[:, 0, :],
                                 start=True, stop=False)
                nc.tensor.matmul(out=ps[:], lhsT=xt[:, 1, :], rhs=wp3[:, 1, :],
                                 start=False, stop=True)
                st = stp.tile([128, TW], F16, tag="st")
                nc.scalar.copy(st[:], ps[:])
                nc.vector.tensor_copy(ssrc_sb.ap()[:, s:s + 1],
                                      ps[:, TW - 1:TW])
                nc.sync.dma_start(shard_d.ap()[128 * s:128 * (s + 1), :], st[:])

            nc.gpsimd.collective_compute(
                "AllGather", mybir.AluOpType.bypass,
                replica_groups=[list(range(NCORES))],
                ins=[shard_d.ap()], outs=[table_d.ap()])

            # phase 2: gather + per-edge weights + weighted segment sum
            out3 = out_d.ap().rearrange("(s p) c -> p s c", p=128)
            for s in range(SUP):
                G = gp.tile([128, DEG, TW], F16, tag="G")
                for t in range(DEG):
                    gi = nc.gpsimd.indirect_dma_start(
                        out=G[:, t, :], out_offset=None,
                        in_=table_d.ap(),
                        in_offset=bass.IndirectOffsetOnAxis(
                            ap=dst_sb.ap()[:, DEG * s + t:DEG * s + t + 1],
                            axis=0))
                    if t // 4:
                        gi.queue = f"qPoolDynamic{t // 4}"

                g_sd = _mkap(G[:], C, [[TW, DEG]])
                sc = sp.tile([128, DEG], F32, tag="sc")
                nc.vector.tensor_scalar(
                    out=sc[:], in0=g_sd,
                    scalar1=ssrc_sb.ap()[:, s:s + 1], scalar2=None,
                    op0=mybir.AluOpType.add)
                lr = sp.tile([128, DEG], F32, tag="lr")
                nc.vector.scalar_tensor_tensor(
                    out=lr[:], in0=sc[:], scalar=NEG_SLOPE, in1=sc[:],
                    op0=mybir.AluOpType.mult, op1=mybir.AluOpType.max)
                wt = sp.tile([128, DEG], F32, tag="wt")
                nc.scalar.activation(
                    wt[:], lr[:], mybir.ActivationFunctionType.Exp,
                    bias=nlnd_sb.ap()[:, s:s + 1])

                prod = pp.tile([128, DEG, C], F32, tag="prod")
                g_sup = _mkap(G[:], 0, [[TW, DEG], [1, C]])
                nc.vector.tensor_tensor(
                    out=prod[:], in0=g_sup,
                    in1=wt[:].to_broadcast([128, DEG, C]),
                    op=mybir.AluOpType.mult)

                ob = obp.tile([128, C], F32, tag="ob")
                red_in = _mkap(prod[:], 0, [[1, C], [C, DEG]])
                nc.vector.tensor_reduce(
                    out=ob[:], in_=red_in,
                    axis=mybir.AxisListType.X, op=mybir.AluOpType.add)
                nc.sync.dma_start(out3[:, s:s + 1, :], ob[:])

    nc.compile()
    return nc


def _host_prep(x, dst, adj_values, weight, attention):
    dst = np.asarray(dst)
    dst_rows = ((dst // NPC) * NPAD + dst % NPC).astype(np.int32)

    weight = np.asarray(weight, np.float32)
    att = np.asarray(attention, np.float32).reshape(2 * C)
    a_src, a_dst = att[:C], att[C:]
    wp = np.empty((IN_CH, TW), np.float32)
    wp[:, :C] = weight
    wp[:, C] = weight @ a_dst
    wp[:, C + 1] = weight @ a_src
    wp = np.ascontiguousarray(wp.astype(np.float16))

    adj = np.asarray(adj_values, np.float32).reshape(N_NODES, DEG)
    deg = adj.sum(axis=1)

    in_maps = []
    for c in range(NCORES):
        xT = np.zeros((IN_CH, NPAD), np.float16)
        xT[:, :NPC] = np.asarray(x[c * NPC:(c + 1) * NPC], np.float32).T
        nlnd = np.full((NPAD,), -np.log(np.float32(DEG)), np.float32)
        nlnd[:NPC] = -np.log(deg[c * NPC:(c + 1) * NPC])
        nlnd = np.ascontiguousarray(nlnd.reshape(SUP, 128).T)
        dr = np.zeros((NPAD, DEG), np.int32)
        dr[:NPC] = dst_rows[c * NPC * DEG:(c + 1) * NPC * DEG].reshape(NPC, DEG)
        dstT = (dr.reshape(SUP, 128, DEG)
                  .transpose(1, 0, 2)
                  .reshape(128, SUP * DEG))
        in_maps.append({
            "xT": xT,
            "dstT": np.ascontiguousarray(dstT),
            "nlnd": nlnd,
            "wp": wp,
        })
    return in_maps


def _numpy_fallback(x, edge_index, adj_values, weight, attention):
    N = x.shape[0]
    x = np.asarray(x, np.float32)
    support = (x @ np.asarray(weight, np.float32)).reshape(N, 1, C)
    src = np.asarray(edge_index[0])
    dst = np.asarray(edge_index[1])
    att = np.asarray(attention, np.float32).reshape(1, 1, 2 * C)
    a_src, a_dst = att[0, :, :C], att[0, :, C:]
    s_src = np.einsum('nhc,hc->nh', support, a_src)
    s_dst = np.einsum('nhc,hc->nh', support, a_dst)
    z = s_src[src] + s_dst[dst]
    edge_e = np.exp(np.where(z >= 0, z, NEG_SLOPE * z))
    deg = np.zeros(N, np.float32)
    np.add.at(deg, src, np.asarray(adj_values, np.float32))
    edge_e = edge_e / deg[src][:, None]
    out = np.zeros((N, 1, C), np.float32)
    np.add.at(out, src, edge_e[:, :, None] * support[dst])
    return out.astype(np.float32)


def kernel(x, edge_index, adj_values, weight, attention):
    global LAST_EXEC_NS, _CACHED_NC
    x = np.asarray(x)
    edge_index = np.asarray(edge_index)
    src = edge_index[0]

    expected_src = np.repeat(
        np.arange(N_NODES, dtype=src.dtype), DEG)
    if x.shape[0] != N_NODES or not np.array_equal(src, expected_src):
        # unexpected structure: fall back to a host reference implementation
        return _numpy_fallback(x, edge_index, adj_values, weight, attention)

    if _CACHED_NC is None:
        _CACHED_NC = _build_nc()
    nc = _CACHED_NC

    in_maps = _host_prep(x, edge_index[1], adj_values, weight, attention)

    trace = os.environ.get("GAT_BASS_TRACE", "") == "1"
    kwargs = {}
    if trace:
        try:
            import prof_hook
            prof_hook.install()
        except Exception:
            trace = False
    res = bass_utils.run_bass_kernel_spmd(
        nc, in_maps, core_ids=list(range(NCORES)), trace=trace)
    LAST_EXEC_NS = res.exec_time_ns

    parts = [res.results[c]["out"][:NPC] for c in range(NCORES)]
    out = np.concatenate(parts, 0).reshape(N_NODES, 1, C)
    return np.ascontiguousarray(out.astype(np.float32))



# revision 8
# speedup vs baseline: 1.5907x; 1.5907x over previous
"""GATConv (nn_GATConv_45595372814934) Trainium2 Bass kernel, 8 NeuronCores.

kernel(**inputs) -> [100000, 1, 64] float32.

Strategy (graph/edge parallelism):
- Node/edge shard: core c owns nodes [12500c, 12500(c+1)) and their 16
  out-edges each (src is repeat(arange(N), 16), so edges are contiguous).
- Phase 1 (per core): support shard = x_c @ W' where W' = [W | W@a_dst |
  W@a_src], fp16 rows [support(64) | s_dst | s_src] written into a
  paired-padded layout (2 rows per 512B block) -> AllGather (2 chunks,
  overlapping phase 1) into a full [50176 pairs, 512B] fp16 table in HBM.
- Phase 2 (per core): ONE batched dma_gather (SWDGE extended inst) per
  128-node supertile fetches all 2048 edge pair-blocks (264B each) by
  int16 pair index (biased by -32768; a trailing positive sentinel index
  defeats the ucode's trailing-negative trim).  Per-edge weights are
  computed for BOTH rows of each gathered pair; the wrong-parity half
  gets -40 added to its leaky-relu score before exp, so its weight
  underflows to zero.  Weighted sum over the 32 half-slots is a single
  fp16 multiply + 5 halving adds (no matmuls, no PSUM in phase 2).
"""

import os
import sys

sys.path.insert(0, "/opt/trn_rl_repo")

import numpy as np

import concourse.bacc as bacc
import concourse.bass as bass
import concourse.mybir as mybir
import concourse.tile as tile
from concourse import bass_utils, library_config
from concourse.bass import AP

F32 = mybir.dt.float32
F16 = mybir.dt.float16
I16 = mybir.dt.int16

N_NODES = 100000
IN_CH = 256
C = 64
DEG = 16
NEG_SLOPE = 0.2
NCORES = 8
NPC = N_NODES // NCORES          # 12500 real nodes per core
NPAD = -(-NPC // 128) * 128      # 12544
SUP = NPAD // 128                # 98 super-tiles
TW = 66                          # table row: support(64) | s_dst | s_src
PB = 256                         # fp16 elements per pair block (512 B)
ELEM = 2 * TW                    # gathered elements per edge (264 B)
NPAIRS_C = NPAD // 2             # 6272 pairs per core shard
NPAIRS = NCORES * NPAIRS_C       # 50176 global pairs
BIAS = 32768                     # idx16 = pair - BIAS
NIDX = 2 * DEG * 64 + 1          # 2049: 2048 edges + 1 positive sentinel
JCOL = 17                        # ceil(2049/128) output columns
ICOL = -(-NIDX // 16) + (16 - 1) // 16  # snake cols: ceil(2176/16)=136
ICOL = 136
HCHUNK = SUP // 2                # 49 supertiles per AllGather chunk

LAST_EXEC_NS = None
_CACHED_NC = None


def _mkap(base: AP, extra_off: int, dims) -> AP:
    return AP(base.tensor, base.offset + extra_off,
              [list(base.ap[0])] + [list(d) for d in dims])


def _dma_gather_raw(nc, out_ap, in_ap, idxs_ap, num_idxs_acc, num_idxs,
                    elem_size, elem_step, queue_num):
    """bass.dma_gather minus the elem_size%256 assert (ucode takes any u16)."""
    g = nc.gpsimd
    stride_bytes = elem_step * mybir.dt.size(in_ap.dtype)
    stride_bytes_256 = stride_bytes // 256
    assert stride_bytes % 256 == 0 and stride_bytes_256 < 256
    _in_ap = g.lower_ap_dma(in_ap, for_custom_bir_dma=True)
    _idxs_ap = g.lower_ap(idxs_ap)
    _out_ap = g.lower_ap(out_ap)
    return g.add_instruction(
        mybir.InstDMAGatherAnt(
            name=nc.get_next_instruction_name(),
            ins=[*_in_ap, _idxs_ap, num_idxs_acc],
            outs=[_out_ap],
            transpose=False,
            num_idxs=num_idxs,
            elem_size=elem_size,
            stride_bytes_256=stride_bytes_256,
            gen_mode=0,
            single_packet=False,
            queue_num=queue_num,
            sbuf_tokens_per_rank=0,
            sbuf_free_dim_per_rank=0,
            sbuf_free_dim_pad_per_rank=0,
            sbuf_byte_offset=0,
        ))


def _build_nc():
    nc = bacc.Bacc("TRN2", target_bir_lowering=False, debug=False,
                   num_devices=NCORES, num_swdge_queues=4)

    xT_d = nc.dram_tensor("xT", [IN_CH, NPAD], F16, kind="ExternalInput")
    idx_d = nc.dram_tensor("idx", [128, SUP * ICOL], I16, kind="ExternalInput")
    par_d = nc.dram_tensor("par", [128, SUP * 2 * DEG], F16,
                           kind="ExternalInput")
    nlnd_d = nc.dram_tensor("nlnd", [128, SUP], F32, kind="ExternalInput")
    wp_d = nc.dram_tensor("wp", [IN_CH, TW], F16, kind="ExternalInput")
    out_d = nc.dram_tensor("out", [NPAD, C], F32, kind="ExternalOutput")

    from concourse.replica_groups import maybe_share_collective_output_space
    aspace = maybe_share_collective_output_space(
        "AllGather", [list(range(NCORES))])
    shard_a = nc.dram_tensor("shard_a", [NPAIRS_C // 2, PB], F16,
                             kind="Internal")
    shard_b = nc.dram_tensor("shard_b", [NPAIRS_C // 2, PB], F16,
                             kind="Internal")
    table_d = nc.dram_tensor("table", [NPAIRS, PB], F16, kind="Internal",
                             addr_space=aspace)

    idx_sb = nc.alloc_sbuf_tensor("idx_sb", [128, SUP * ICOL], I16)
    par_sb = nc.alloc_sbuf_tensor("par_sb", [128, SUP * 2 * DEG], F16)
    ssrc_sb = nc.alloc_sbuf_tensor("ssrc_sb", [128, SUP], F32)
    nlnd_sb = nc.alloc_sbuf_tensor("nlnd_sb", [128, SUP], F32)
    wp_sb = nc.alloc_sbuf_tensor("wp_sb", [128, 2 * TW], F16)

    with tile.TileContext(nc) as tc:
        with (
            tc.tile_pool(name="xp", bufs=3) as xp,
            tc.tile_pool(name="stp", bufs=3) as stp,
            tc.tile_pool(name="gp", bufs=4) as gp,
            tc.tile_pool(name="sp", bufs=3) as sp,
            tc.tile_pool(name="pp", bufs=3) as pp,
            tc.tile_pool(name="hp", bufs=2) as hp,
            tc.tile_pool(name="obp", bufs=3) as obp,
            tc.tile_pool(name="ps1", bufs=2, space="PSUM") as ps1,
        ):
            nc.gpsimd.load_library(library_config.mlp)
            nc.sync.dma_start(idx_sb.ap(), idx_d.ap())
            nc.sync.dma_start(par_sb.ap(), par_d.ap())
            nc.sync.dma_start(nlnd_sb.ap(), nlnd_d.ap())
            nc.sync.dma_start(
                wp_sb.ap(), wp_d.ap().rearrange("(a p) c -> p a c", p=128))
            wp3 = wp_sb.ap().rearrange("p (a c) -> p a c", c=TW)
            nreg = nc.gpsimd.lower_val_access(nc.gpsimd.to_reg(NIDX))

            # phase 1: support table shard in paired-padded layout
            xT3 = xT_d.ap().rearrange("(a p) n -> p a n", p=128)
            for s in range(SUP):
                xt = xp.tile([128, 2, 128], F16, tag="xt")
                nc.sync.dma_start(xt[:], xT3[:, :, 128 * s:128 * (s + 1)])
                ps = ps1.tile([128, TW], F32, tag="ps1")
                nc.tensor.matmul(out=ps[:], lhsT=xt[:, 0, :], rhs=wp3[:, 0, :],
                                 start=True, stop=False)
                nc.tensor.matmul(out=ps[:], lhsT=xt[:, 1, :], rhs=wp3[:, 1, :],
                                 start=False, stop=True)
                st = stp.tile([128, TW], F16, tag="st")
                nc.scalar.copy(st[:], ps[:])
                nc.vector.tensor_copy(ssrc_sb.ap()[:, s:s + 1],
                                      ps[:, TW - 1:TW])
                shard = shard_a if s < HCHUNK else shard_b
                soff = (s if s < HCHUNK else s - HCHUNK) * 64 * PB
                dst = AP(shard.ap().tensor, soff,
                         [[PB, 64], [TW, 2], [1, TW]])
                nc.scalar.dma_start(dst, st[:])

            # AllGather in 2 chunks; chunk 0 overlaps phase-1 second half.
            # Table layout is [chunk, core, pairs] so each chunk's output is
            # one contiguous slice (collectives require contiguous outs).
            half = NCORES * (NPAIRS_C // 2)
            for k, shard in enumerate((shard_a, shard_b)):
                nc.gpsimd.collective_compute(
                    "AllGather", mybir.AluOpType.bypass,
                    replica_groups=[list(range(NCORES))],
                    ins=[shard.ap()],
                    outs=[table_d.ap()[k * half:(k + 1) * half, :]])

            # phase 2: batched pair-gather + parity-masked weights + reduce
            out3 = out_d.ap().rearrange("(s p) c -> p s c", p=128)
            tb_ap = table_d.ap()[BIAS:, :ELEM]
            for s in range(SUP):
                G = gp.tile([128, JCOL, ELEM], F16, tag="G")
                _dma_gather_raw(
                    nc, G[:], tb_ap,
                    idx_sb.ap()[:, ICOL * s:ICOL * (s + 1)],
                    nreg, NIDX, ELEM, PB, queue_num=s % 4)

                # scores for both halves of each pair: z = s_dst + s_src
                g_sd = _mkap(G[:], C, [[TW, 2 * DEG]])
                z2 = sp.tile([128, 2 * DEG], F32, tag="z2")
                nc.vector.tensor_scalar(
                    out=z2[:], in0=g_sd,
                    scalar1=ssrc_sb.ap()[:, s:s + 1], scalar2=None,
                    op0=mybir.AluOpType.add)
                lr = sp.tile([128, 2 * DEG], F32, tag="lr")
                nc.vector.scalar_tensor_tensor(
                    out=lr[:], in0=z2[:], scalar=NEG_SLOPE, in1=z2[:],
                    op0=mybir.AluOpType.mult, op1=mybir.AluOpType.max)
                # wrong-parity half gets -40 => exp underflows to 0
                lrm = sp.tile([128, 2 * DEG], F32, tag="lrm")
                nc.vector.tensor_tensor(
                    out=lrm[:], in0=lr[:],
                    in1=par_sb.ap()[:, 2 * DEG * s:2 * DEG * (s + 1)],
                    op=mybir.AluOpType.add)
                wt = sp.tile([128, 2 * DEG], F16, tag="wt")
                nc.scalar.activation(
                    wt[:], lrm[:], mybir.ActivationFunctionType.Exp,
                    bias=nlnd_sb.ap()[:, s:s + 1])

                g_sup = _mkap(G[:], 0, [[TW, 2 * DEG], [1, C]])
                prod = pp.tile([128, 2 * DEG, C], F16, tag="prod")
                nc.vector.tensor_tensor(
                    out=prod[:], in0=g_sup,
                    in1=wt[:].to_broadcast([128, 2 * DEG, C]),
                    op=mybir.AluOpType.mult)

                h1 = hp.tile([128, DEG, C], F16, tag="h1")
                nc.vector.tensor_tensor(out=h1[:], in0=prod[:, :DEG, :],
                                        in1=prod[:, DEG:, :],
                                        op=mybir.AluOpType.add)
                h2 = hp.tile([128, DEG // 2, C], F16, tag="h2")
                nc.vector.tensor_tensor(out=h2[:], in0=h1[:, :DEG // 2, :],
                                        in1=h1[:, DEG // 2:, :],
                                        op=mybir.AluOpType.add)
                h3 = hp.tile([128, DEG // 4, C], F16, tag="h3")
                nc.vector.tensor_tensor(out=h3[:], in0=h2[:, :DEG // 4, :],
                                        in1=h2[:, DEG // 4:, :],
                                        op=mybir.AluOpType.add)
                h4 = hp.tile([128, 2, C], F16, tag="h4")
                nc.vector.tensor_tensor(out=h4[:], in0=h3[:, :2, :],
                                        in1=h3[:, 2:, :],
                                        op=mybir.AluOpType.add)
                ob = obp.tile([128, C], F32, tag="ob")
                nc.vector.tensor_tensor(out=ob[:], in0=h4[:, 0, :],
                                        in1=h4[:, 1, :],
                                        op=mybir.AluOpType.add)
                nc.scalar.dma_start(out3[:, s:s + 1, :], ob[:])

    nc.compile()
    return nc


def _host_prep(x, dst, adj_values, weight, attention):
    dst = np.asarray(dst)
    core = (dst // NPC).astype(np.int32)
    local = (dst % NPC).astype(np.int32)
    ql = local >> 1                      # local pair within core shard
    chunk = (ql >= NPAIRS_C // 2).astype(np.int32)
    # table layout [chunk, core, pairs]: global pair id
    q_all = (chunk * (NCORES * (NPAIRS_C // 2)) + core * (NPAIRS_C // 2)
             + (ql - chunk * (NPAIRS_C // 2))).astype(np.int32)
    p_all = (local & 1).astype(np.int32)

    weight = np.asarray(weight, np.float32)
    att = np.asarray(attention, np.float32).reshape(2 * C)
    a_src, a_dst = att[:C], att[C:]
    wp = np.empty((IN_CH, TW), np.float32)
    wp[:, :C] = weight
    wp[:, C] = weight @ a_dst
    wp[:, C + 1] = weight @ a_src
    wp = np.ascontiguousarray(wp.astype(np.float16))

    adj = np.asarray(adj_values, np.float32).reshape(N_NODES, DEG)
    deg = adj.sum(axis=1)

    in_maps = []
    for c in range(NCORES):
        xT = np.zeros((IN_CH, NPAD), np.float16)
        xT[:, :NPC] = np.asarray(x[c * NPC:(c + 1) * NPC], np.float32).T
        nlnd = np.full((NPAD,), -np.log(np.float32(DEG)), np.float32)
        nlnd[:NPC] = -np.log(deg[c * NPC:(c + 1) * NPC])
        nlnd = np.ascontiguousarray(nlnd.reshape(SUP, 128).T)

        qc = np.full((NPAD, DEG), BIAS, np.int32)
        pc = np.zeros((NPAD, DEG), np.int32)
        sl = slice(c * NPC * DEG, (c + 1) * NPC * DEG)
        qc[:NPC] = q_all[sl].reshape(NPC, DEG)
        pc[:NPC] = p_all[sl].reshape(NPC, DEG)

        # idx snake: per supertile s, logical index k=j*128+p -> value
        # qc[128s+p, j]-BIAS at snake position [k%16 (replicated x8), k//16]
        idx_k = (qc.reshape(SUP, 128, DEG).transpose(0, 2, 1)
                   .reshape(SUP, 2 * DEG * 64) - BIAS).astype(np.int16)
        snake = np.zeros((SUP, 16, ICOL), np.int16)
        snake[:, :, :128] = idx_k.reshape(SUP, 128, 16).transpose(0, 2, 1)
        idx16 = np.tile(
            snake.transpose(1, 0, 2).reshape(16, SUP * ICOL), (8, 1))

        # parity shift: 0 where half h matches edge parity, -40 otherwise
        par2 = np.full((SUP, 128, DEG, 2), np.float16(-40.0), np.float16)
        pcs = pc.reshape(SUP, 128, DEG)
        one = np.arange(2)[None, None, None, :] == pcs[..., None]
        par2[one] = np.float16(0.0)
        par = np.ascontiguousarray(
            par2.reshape(SUP, 128, 2 * DEG).transpose(1, 0, 2)
                .reshape(128, SUP * 2 * DEG))

        in_maps.append({
            "xT": xT,
            "idx": np.ascontiguousarray(idx16),
            "par": par,
            "nlnd": nlnd,
            "wp": wp,
        })
    return in_maps


def _numpy_fallback(x, edge_index, adj_values, weight, attention):
    N = x.shape[0]
    x = np.asarray(x, np.float32)
    support = (x @ np.asarray(weight, np.float32)).reshape(N, 1, C)
    src = np.asarray(edge_index[0])
    dst = np.asarray(edge_index[1])
    att = np.asarray(attention, np.float32).reshape(1, 1, 2 * C)
    a_src, a_dst = att[0, :, :C], att[0, :, C:]
    s_src = np.einsum('nhc,hc->nh', support, a_src)
    s_dst = np.einsum('nhc,hc->nh', support, a_dst)
    z = s_src[src] + s_dst[dst]
    edge_e = np.exp(np.where(z >= 0, z, NEG_SLOPE * z))
    deg = np.zeros(N, np.float32)
    np.add.at(deg, src, np.asarray(adj_values, np.float32))
    edge_e = edge_e / deg[src][:, None]
    out = np.zeros((N, 1, C), np.float32)
    np.add.at(out, src, edge_e[:, :, None] * support[dst])
    return out.astype(np.float32)


def kernel(x, edge_index, adj_values, weight, attention):
    global LAST_EXEC_NS, _CACHED_NC
    x = np.asarray(x)
    edge_index = np.asarray(edge_index)
    src = edge_index[0]

    expected_src = np.repeat(
        np.arange(N_NODES, dtype=src.dtype), DEG)
    if x.shape[0] != N_NODES or not np.array_equal(src, expected_src):
        # unexpected structure: fall back to a host reference implementation
        return _numpy_fallback(x, edge_index, adj_values, weight, attention)

    if _CACHED_NC is None:
        _CACHED_NC = _build_nc()
    nc = _CACHED_NC

    in_maps = _host_prep(x, edge_index[1], adj_values, weight, attention)

    trace = os.environ.get("GAT_BASS_TRACE", "") == "1"
    kwargs = {}
    if trace:
        try:
            import prof_hook
            prof_hook.install()
        except Exception:
            trace = False
    res = bass_utils.run_bass_kernel_spmd(
        nc, in_maps, core_ids=list(range(NCORES)), trace=trace)
    LAST_EXEC_NS = res.exec_time_ns

    parts = [res.results[c]["out"][:NPC] for c in range(NCORES)]
    out = np.concatenate(parts, 0).reshape(N_NODES, 1, C)
    return np.ascontiguousarray(out.astype(np.float32))


# revision 11
# speedup vs baseline: 1.6419x; 1.0322x over previous
"""GATConv (nn_GATConv_45595372814934) Trainium2 Bass kernel, 8 NeuronCores.

kernel(**inputs) -> [100000, 1, 64] float32.

Strategy (graph/edge parallelism):
- Node/edge shard: core c owns nodes [12500c, 12500(c+1)) and their 16
  out-edges each (src is repeat(arange(N), 16), so edges are contiguous).
- Phase 1 (per core): support shard = x_c @ W' where W' = [W | W@a_dst |
  W@a_src], fp16 rows [support(64) | s_dst | s_src] written into a
  paired-padded layout (2 rows per 512B block) -> AllGather (2 chunks,
  overlapping phase 1) into a full [50176 pairs, 512B] fp16 table in HBM.
- Phase 2 (per core): ONE batched dma_gather (SWDGE extended inst) per
  128-node supertile fetches all 2048 edge pair-blocks (264B each) by
  int16 pair index (biased by -17408; each node's 16 edges are slot-
  ordered by ascending pair id so the snake's last position holds a
  max-q index, defeating the ucode's trailing-negative trim).  Per-edge weights are
  computed for BOTH rows of each gathered pair; the wrong-parity half
  gets -40 added to its leaky-relu score before exp, so its weight
  underflows to zero.  Weighted sum over the 32 half-slots is a single
  fp16 multiply + 5 halving adds (no matmuls, no PSUM in phase 2).
"""

import os
import sys

sys.path.insert(0, "/opt/trn_rl_repo")

import numpy as np

import concourse.bacc as bacc
import concourse.bass as bass
import concourse.mybir as mybir
import concourse.tile as tile
from concourse import bass_utils, library_config
from concourse.bass import AP

F32 = mybir.dt.float32
F16 = mybir.dt.float16
I16 = mybir.dt.int16

N_NODES = 100000
IN_CH = 256
C = 64
DEG = 16
NEG_SLOPE = 0.2
NCORES = 8
NPC = N_NODES // NCORES          # 12500 real nodes per core
NPAD = -(-NPC // 128) * 128      # 12544
SUP = NPAD // 128                # 98 super-tiles
TW = 66                          # table row: support(64) | s_dst | s_src
PB = 256                         # fp16 elements per pair block (512 B)
ELEM = 2 * TW                    # gathered elements per edge (264 B)
NPAIRS_C = NPAD // 2             # 6272 pairs per core shard
NPAIRS = NCORES * NPAIRS_C       # 50176 global pairs
BIAS = 17408                     # idx16 = pair - BIAS (range [-17408, 32767])
NIDX = 2 * DEG * 64              # 2048 edges, no sentinel: per-node ascending-q
                                 # order puts a (w.h.p. non-negative) max-q idx
                                 # at position 2047, defeating trailing trim
JCOL = 16                        # output columns
ICOL = 128                       # snake cols: 2048/16
HCHUNK = SUP // 2                # 49 supertiles per AllGather chunk

LAST_EXEC_NS = None
_CACHED_NC = None


def _mkap(base: AP, extra_off: int, dims) -> AP:
    return AP(base.tensor, base.offset + extra_off,
              [list(base.ap[0])] + [list(d) for d in dims])


def _dma_gather_raw(nc, out_ap, in_ap, idxs_ap, num_idxs_acc, num_idxs,
                    elem_size, elem_step, queue_num):
    """bass.dma_gather minus the elem_size%256 assert (ucode takes any u16)."""
    g = nc.gpsimd
    stride_bytes = elem_step * mybir.dt.size(in_ap.dtype)
    stride_bytes_256 = stride_bytes // 256
    assert stride_bytes % 256 == 0 and stride_bytes_256 < 256
    _in_ap = g.lower_ap_dma(in_ap, for_custom_bir_dma=True)
    _idxs_ap = g.lower_ap(idxs_ap)
    _out_ap = g.lower_ap(out_ap)
    return g.add_instruction(
        mybir.InstDMAGatherAnt(
            name=nc.get_next_instruction_name(),
            ins=[*_in_ap, _idxs_ap, num_idxs_acc],
            outs=[_out_ap],
            transpose=False,
            num_idxs=num_idxs,
            elem_size=elem_size,
            stride_bytes_256=stride_bytes_256,
            gen_mode=0,
            single_packet=False,
            queue_num=queue_num,
            sbuf_tokens_per_rank=0,
            sbuf_free_dim_per_rank=0,
            sbuf_free_dim_pad_per_rank=0,
            sbuf_byte_offset=0,
        ))


def _build_nc():
    nc = bacc.Bacc("TRN2", target_bir_lowering=False, debug=False,
                   num_devices=NCORES, num_swdge_queues=4)

    xT_d = nc.dram_tensor("xT", [IN_CH, NPAD], F16, kind="ExternalInput")
    idx_d = nc.dram_tensor("idx", [128, SUP * ICOL], I16, kind="ExternalInput")
    par_d = nc.dram_tensor("par", [128, SUP * 2 * DEG], F16,
                           kind="ExternalInput")
    nlnd_d = nc.dram_tensor("nlnd", [128, SUP], F32, kind="ExternalInput")
    wp_d = nc.dram_tensor("wp", [IN_CH, TW], F16, kind="ExternalInput")
    out_d = nc.dram_tensor("out", [NPAD, C], F32, kind="ExternalOutput")

    from concourse.replica_groups import maybe_share_collective_output_space
    aspace = maybe_share_collective_output_space(
        "AllGather", [list(range(NCORES))])
    shard_a = nc.dram_tensor("shard_a", [NPAIRS_C // 2, PB], F16,
                             kind="Internal")
    shard_b = nc.dram_tensor("shard_b", [NPAIRS_C // 2, PB], F16,
                             kind="Internal")
    table_d = nc.dram_tensor("table", [NPAIRS, PB], F16, kind="Internal",
                             addr_space=aspace)

    idx_sb = nc.alloc_sbuf_tensor("idx_sb", [128, SUP * ICOL], I16)
    par_sb = nc.alloc_sbuf_tensor("par_sb", [128, SUP * 2 * DEG], F16)
    ssrc_sb = nc.alloc_sbuf_tensor("ssrc_sb", [128, SUP], F32)
    nlnd_sb = nc.alloc_sbuf_tensor("nlnd_sb", [128, SUP], F32)
    wp_sb = nc.alloc_sbuf_tensor("wp_sb", [128, 2 * TW], F16)

    with tile.TileContext(nc) as tc:
        with (
            tc.tile_pool(name="xp", bufs=3) as xp,
            tc.tile_pool(name="stp", bufs=3) as stp,
            tc.tile_pool(name="gp", bufs=4) as gp,
            tc.tile_pool(name="sp", bufs=3) as sp,
            tc.tile_pool(name="pp", bufs=3) as pp,
            tc.tile_pool(name="hp", bufs=2) as hp,
            tc.tile_pool(name="obp", bufs=3) as obp,
            tc.tile_pool(name="ps1", bufs=2, space="PSUM") as ps1,
        ):
            nc.gpsimd.load_library(library_config.mlp)
            nc.sync.dma_start(idx_sb.ap(), idx_d.ap())
            nc.sync.dma_start(par_sb.ap(), par_d.ap())
            nc.sync.dma_start(nlnd_sb.ap(), nlnd_d.ap())
            nc.sync.dma_start(
                wp_sb.ap(), wp_d.ap().rearrange("(a p) c -> p a c", p=128))
            wp3 = wp_sb.ap().rearrange("p (a c) -> p a c", c=TW)
            nreg = nc.gpsimd.lower_val_access(nc.gpsimd.to_reg(NIDX))

            # phase 1: support table shard in paired-padded layout
            xT3 = xT_d.ap().rearrange("(a p) n -> p a n", p=128)
            for s in range(SUP):
                xt = xp.tile([128, 2, 128], F16, tag="xt")
                nc.sync.dma_start(xt[:], xT3[:, :, 128 * s:128 * (s + 1)])
                ps = ps1.tile([128, TW], F32, tag="ps1")
                nc.tensor.matmul(out=ps[:], lhsT=xt[:, 0, :], rhs=wp3[:, 0, :],
                                 start=True, stop=False)
                nc.tensor.matmul(out=ps[:], lhsT=xt[:, 1, :], rhs=wp3[:, 1, :],
                                 start=False, stop=True)
                st = stp.tile([128, TW], F16, tag="st")
                nc.scalar.copy(st[:], ps[:])
                nc.vector.tensor_copy(ssrc_sb.ap()[:, s:s + 1],
                                      ps[:, TW - 1:TW])
                shard = shard_a if s < HCHUNK else shard_b
                soff = (s if s < HCHUNK else s - HCHUNK) * 64 * PB
                dst = AP(shard.ap().tensor, soff,
                         [[PB, 64], [TW, 2], [1, TW]])
                nc.scalar.dma_start(dst, st[:])

            # AllGather in 2 chunks; chunk 0 overlaps phase-1 second half.
            # Table layout is [chunk, core, pairs] so each chunk's output is
            # one contiguous slice (collectives require contiguous outs).
            half = NCORES * (NPAIRS_C // 2)
            for k, shard in enumerate((shard_a, shard_b)):
                nc.gpsimd.collective_compute(
                    "AllGather", mybir.AluOpType.bypass,
                    replica_groups=[list(range(NCORES))],
                    ins=[shard.ap()],
                    outs=[table_d.ap()[k * half:(k + 1) * half, :]])

            # phase 2: batched pair-gather + parity-masked weights + reduce
            out3 = out_d.ap().rearrange("(s p) c -> p s c", p=128)
            tb_ap = table_d.ap()[BIAS:, :ELEM]
            for s in range(SUP):
                G = gp.tile([128, JCOL, ELEM], F16, tag="G")
                _dma_gather_raw(
                    nc, G[:], tb_ap,
                    idx_sb.ap()[:, ICOL * s:ICOL * (s + 1)],
                    nreg, NIDX, ELEM, PB, queue_num=s % 4)

                # scores for both halves of each pair: z = s_dst + s_src
                g_sd = _mkap(G[:], C, [[TW, 2 * DEG]])
                z2 = sp.tile([128, 2 * DEG], F32, tag="z2")
                nc.vector.tensor_scalar(
                    out=z2[:], in0=g_sd,
                    scalar1=ssrc_sb.ap()[:, s:s + 1], scalar2=None,
                    op0=mybir.AluOpType.add)
                lr = sp.tile([128, 2 * DEG], F32, tag="lr")
                nc.vector.scalar_tensor_tensor(
                    out=lr[:], in0=z2[:], scalar=NEG_SLOPE, in1=z2[:],
                    op0=mybir.AluOpType.mult, op1=mybir.AluOpType.max)
                # wrong-parity half gets -40 => exp underflows to 0
                lrm = sp.tile([128, 2 * DEG], F32, tag="lrm")
                nc.vector.tensor_tensor(
                    out=lrm[:], in0=lr[:],
                    in1=par_sb.ap()[:, 2 * DEG * s:2 * DEG * (s + 1)],
                    op=mybir.AluOpType.add)
                wt = sp.tile([128, 2 * DEG], F16, tag="wt")
                nc.scalar.activation(
                    wt[:], lrm[:], mybir.ActivationFunctionType.Exp,
                    bias=nlnd_sb.ap()[:, s:s + 1])

                g_sup = _mkap(G[:], 0, [[TW, 2 * DEG], [1, C]])
                prod = pp.tile([128, 2 * DEG, C], F16, tag="prod")
                nc.vector.tensor_tensor(
                    out=prod[:], in0=g_sup,
                    in1=wt[:].to_broadcast([128, 2 * DEG, C]),
                    op=mybir.AluOpType.mult)

                h1 = hp.tile([128, DEG, C], F16, tag="h1")
                nc.vector.tensor_tensor(out=h1[:], in0=prod[:, :DEG, :],
                                        in1=prod[:, DEG:, :],
                                        op=mybir.AluOpType.add)
                h2 = hp.tile([128, DEG // 2, C], F16, tag="h2")
                nc.vector.tensor_tensor(out=h2[:], in0=h1[:, :DEG // 2, :],
                                        in1=h1[:, DEG // 2:, :],
                                        op=mybir.AluOpType.add)
                h3 = hp.tile([128, DEG // 4, C], F16, tag="h3")
                nc.vector.tensor_tensor(out=h3[:], in0=h2[:, :DEG // 4, :],
                                        in1=h2[:, DEG // 4:, :],
                                        op=mybir.AluOpType.add)
                h4 = hp.tile([128, 2, C], F16, tag="h4")
                nc.vector.tensor_tensor(out=h4[:], in0=h3[:, :2, :],
                                        in1=h3[:, 2:, :],
                                        op=mybir.AluOpType.add)
                ob = obp.tile([128, C], F32, tag="ob")
                nc.vector.tensor_tensor(out=ob[:], in0=h4[:, 0, :],
                                        in1=h4[:, 1, :],
                                        op=mybir.AluOpType.add)
                nc.scalar.dma_start(out3[:, s:s + 1, :], ob[:])

    nc.compile()
    return nc


def _host_prep(x, dst, adj_values, weight, attention):
    dst = np.asarray(dst)
    core = (dst // NPC).astype(np.int32)
    local = (dst % NPC).astype(np.int32)
    ql = local >> 1                      # local pair within core shard
    chunk = (ql >= NPAIRS_C // 2).astype(np.int32)
    # table layout [chunk, core, pairs]: global pair id
    q_all = (chunk * (NCORES * (NPAIRS_C // 2)) + core * (NPAIRS_C // 2)
             + (ql - chunk * (NPAIRS_C // 2))).astype(np.int32)
    p_all = (local & 1).astype(np.int32)

    weight = np.asarray(weight, np.float32)
    att = np.asarray(attention, np.float32).reshape(2 * C)
    a_src, a_dst = att[:C], att[C:]
    wp = np.empty((IN_CH, TW), np.float32)
    wp[:, :C] = weight
    wp[:, C] = weight @ a_dst
    wp[:, C + 1] = weight @ a_src
    wp = np.ascontiguousarray(wp.astype(np.float16))

    adj = np.asarray(adj_values, np.float32).reshape(N_NODES, DEG)
    deg = adj.sum(axis=1)

    in_maps = []
    for c in range(NCORES):
        xT = np.zeros((IN_CH, NPAD), np.float16)
        xT[:, :NPC] = np.asarray(x[c * NPC:(c + 1) * NPC], np.float32).T
        nlnd = np.full((NPAD,), -np.log(np.float32(DEG)), np.float32)
        nlnd[:NPC] = -np.log(deg[c * NPC:(c + 1) * NPC])
        nlnd = np.ascontiguousarray(nlnd.reshape(SUP, 128).T)

        qc = np.full((NPAD, DEG), BIAS, np.int32)
        pc = np.zeros((NPAD, DEG), np.int32)
        sl = slice(c * NPC * DEG, (c + 1) * NPC * DEG)
        qc[:NPC] = q_all[sl].reshape(NPC, DEG)
        pc[:NPC] = p_all[sl].reshape(NPC, DEG)

        # per-node ascending-q slot order (slot 15 = max q): the snake's
        # last position (node 128s+127, slot 15) is then non-negative after
        # bias w.h.p., so the ucode's trailing-negative trim never fires.
        order = np.argsort(qc, axis=1, kind="stable")
        qc = np.take_along_axis(qc, order, axis=1)
        pc = np.take_along_axis(pc, order, axis=1)
        if (qc[127::128, DEG - 1] < BIAS).any():
            return None  # pathological input: caller falls back to numpy

        # idx snake: per supertile s, logical index k=j*128+p -> value
        # qc[128s+p, j]-BIAS at snake position [k%16 (replicated x8), k//16]
        idx_k = (qc.reshape(SUP, 128, DEG).transpose(0, 2, 1)
                   .reshape(SUP, 2 * DEG * 64) - BIAS).astype(np.int16)
        snake = idx_k.reshape(SUP, 128, 16).transpose(0, 2, 1)
        idx16 = np.tile(
            snake.transpose(1, 0, 2).reshape(16, SUP * ICOL), (8, 1))

        # parity shift: 0 where half h matches edge parity, -40 otherwise
        par2 = np.full((SUP, 128, DEG, 2), np.float16(-40.0), np.float16)
        pcs = pc.reshape(SUP, 128, DEG)
        one = np.arange(2)[None, None, None, :] == pcs[..., None]
        par2[one] = np.float16(0.0)
        par = np.ascontiguousarray(
            par2.reshape(SUP, 128, 2 * DEG).transpose(1, 0, 2)
                .reshape(128, SUP * 2 * DEG))

        in_maps.append({
            "xT": xT,
            "idx": np.ascontiguousarray(idx16),
            "par": par,
            "nlnd": nlnd,
            "wp": wp,
        })
    return in_maps


def _numpy_fallback(x, edge_index, adj_values, weight, attention):
    N = x.shape[0]
    x = np.asarray(x, np.float32)
    support = (x @ np.asarray(weight, np.float32)).reshape(N, 1, C)
    src = np.asarray(edge_index[0])
    dst = np.asarray(edge_index[1])
    att = np.asarray(attention, np.float32).reshape(1, 1, 2 * C)
    a_src, a_dst = att[0, :, :C], att[0, :, C:]
    s_src = np.einsum('nhc,hc->nh', support, a_src)
    s_dst = np.einsum('nhc,hc->nh', support, a_dst)
    z = s_src[src] + s_dst[dst]
    edge_e = np.exp(np.where(z >= 0, z, NEG_SLOPE * z))
    deg = np.zeros(N, np.float32)
    np.add.at(deg, src, np.asarray(adj_values, np.float32))
    edge_e = edge_e / deg[src][:, None]
    out = np.zeros((N, 1, C), np.float32)
    np.add.at(out, src, edge_e[:, :, None] * support[dst])
    return out.astype(np.float32)


def kernel(x, edge_index, adj_values, weight, attention):
    global LAST_EXEC_NS, _CACHED_NC
    x = np.asarray(x)
    edge_index = np.asarray(edge_index)
    src = edge_index[0]

    expected_src = np.repeat(
        np.arange(N_NODES, dtype=src.dtype), DEG)
    if x.shape[0] != N_NODES or not np.array_equal(src, expected_src):
        # unexpected structure: fall back to a host reference implementation
        return _numpy_fallback(x, edge_index, adj_values, weight, attention)

    if _CACHED_NC is None:
        _CACHED_NC = _build_nc()
    nc = _CACHED_NC

    in_maps = _host_prep(x, edge_index[1], adj_values, weight, attention)
    if in_maps is None:
        return _numpy_fallback(x, edge_index, adj_values, weight, attention)

    trace = os.environ.get("GAT_BASS_TRACE", "") == "1"
    kwargs = {}
    if trace:
        try:
            import prof_hook
            prof_hook.install()
        except Exception:
            trace = False
    res = bass_utils.run_bass_kernel_spmd(
        nc, in_maps, core_ids=list(range(NCORES)), trace=trace)
    LAST_EXEC_NS = res.exec_time_ns

    parts = [res.results[c]["out"][:NPC] for c in range(NCORES)]
    out = np.concatenate(parts, 0).reshape(N_NODES, 1, C)
    return np.ascontiguousarray(out.astype(np.float32))
